# revision 3
# baseline (speedup 1.0000x reference)
"""ROIPool (adaptive max pool over ROI crops) for Trainium2, 8-core SPMD.

Strategy:
  - Host computes every ROI's crop geometry and adaptive-pool bin boundaries
    from the actual inputs (floor/ceil in float32, exactly as the reference).
  - ROIs are sorted by source image and dealt to the 8 cores in contiguous,
    cost-balanced chunks, so each core touches only 1-2 source images.
  - One SPMD Bass program is compiled per call.  Per-core work is baked as
    static instruction streams inside an If/Else-if cascade on
    partition_id (untaken blocks are jumped over entirely):
    each core DMAs its source image(s) into SBUF once ([128 partitions, 2
    channel-halves, 56*56], in 4 row-chunks so compute starts early) and
    then runs exact statically-sliced reduce_max instructions per ROI:
       row phase: for each run of row-bins with equal size+stride, one DVE
                  reduce over the h axis -> rowp[q, i, w_rel]
       col phase: for each run of col-bins, one DVE reduce over w -> out
    Outputs are staged 8 ROIs at a time (ring of 4 staging tiles) and
    DMA'd to DRAM.
  - Max in f32 is exact, so the result is bitwise identical to the
    reference regardless of reduction order.
"""

import numpy as np

OUT = 7
NCORES = 8
FIXED_ELEMS = 200  # ~fixed ns per DVE instr, in element-equivalents


# ---------------------------------------------------------------- host math
def _roi_geometry(images_shape, rois, roi_idx):
    """Replicates the reference's f32 floor/ceil box math on the host."""
    n, c, h, w = images_shape
    rois = np.asarray(rois, dtype=np.float32)
    roi_idx = np.asarray(roi_idx).astype(np.int64)
    x1 = np.floor(rois[:, 0] * np.float32(w)).astype(np.int64)
    y1 = np.floor(rois[:, 1] * np.float32(h)).astype(np.int64)
    x2 = np.ceil(rois[:, 2] * np.float32(w)).astype(np.int64)
    y2 = np.ceil(rois[:, 3] * np.float32(h)).astype(np.int64)
    return x1, y1, x2, y2, roi_idx


def _bins(start, length):
    """Adaptive pool bins [start + i*L//OUT, start + ceil((i+1)*L/OUT)) ."""
    out = []
    for i in range(OUT):
        s = start + (i * length) // OUT
        e = start + ((i + 1) * length + OUT - 1) // OUT
        out.append((int(s), int(e)))
    return out


def _runs(bins):
    """Group consecutive bins into maximal runs with equal size and equal
    start stride.  Returns list of (i0, count, s0, stride, size)."""
    runs = []
    i = 0
    while i < OUT:
        s0, e0 = bins[i]
        size = e0 - s0
        j = i + 1
        stride = None
        while j < OUT:
            s, e = bins[j]
            if e - s != size:
                break
            st = s - bins[j - 1][0]
            if stride is None:
                stride = st
            elif st != stride:
                break
            j += 1
        runs.append((i, j - i, s0, 0 if stride is None else stride, size))
        i = j
    return runs


def _roi_cost(Hr, Wr, y1):
    """Estimated DVE cost (element-equivalents) of one ROI."""
    if Hr <= 0 or Wr <= 0:
        return FIXED_ELEMS
    rr = _runs(_bins(y1, Hr))
    cr = _runs(_bins(0, Wr))
    row = sum(2 * cnt * sz * Wr for (_, cnt, _, _, sz) in rr)
    col = sum(2 * cnt * OUT * sz for (_, cnt, _, _, sz) in cr)
    return row + col + FIXED_ELEMS * (len(rr) + len(cr))


def _balanced_split(costs, k):
    """Split range(len(costs)) into k contiguous chunks with near-equal
    total cost (prefix-sum fractional boundaries)."""
    c = np.asarray(costs, dtype=np.float64)
    pref = np.concatenate([[0.0], np.cumsum(c)])
    total = pref[-1]
    bounds = [0]
    for i in range(1, k):
        bounds.append(int(np.searchsorted(pref, total * i / k)))
    bounds.append(len(costs))
    # enforce monotonicity
    for i in range(1, len(bounds)):
        bounds[i] = max(bounds[i], bounds[i - 1])
    return [(bounds[i], bounds[i + 1]) for i in range(k)]


# ---------------------------------------------------------------- kernel
def kernel(images, rois, roi_idx):
    import concourse.bacc as bacc
    import concourse.bass as bass
    import concourse.mybir as mybir
    from concourse.bass_utils import run_bass_kernel_spmd
    from concourse.tile import TileContext

    images = np.ascontiguousarray(np.asarray(images, dtype=np.float32))
    N, C, H, W = images.shape
    R = np.asarray(rois).shape[0]
    assert C == 256 and H == 56 and W == 56, "kernel hardcoded for C=256, H=W=56"
    HW = H * W
    x1, y1, x2, y2, ridx = _roi_geometry(images.shape, rois, roi_idx)

    # order ROIs by source image (then by bottom row, so earlier image
    # row-chunks unblock the first ROIs), cost-balanced contiguous chunks
    order = sorted(range(R), key=lambda r: (int(ridx[r]), int(y2[r]), r))
    costs = [_roi_cost(int(y2[r] - y1[r]), int(x2[r] - x1[r]), int(y1[r])) for r in order]
    chunks = [np.array(order[a:b], dtype=np.int64) for a, b in _balanced_split(costs, NCORES)]
    nr_max = max(len(ch) for ch in chunks)

    nc = bacc.Bacc("TRN2", target_bir_lowering=False, debug=False, num_devices=NCORES)
    images_d = nc.dram_tensor("images", [N, C, H, W], mybir.dt.float32, kind="ExternalInput")
    out_d = nc.dram_tensor("out", [max(nr_max, 1), C, OUT, OUT], mybir.dt.float32, kind="ExternalOutput")

    NEG = float(np.finfo(np.float32).min)
    NSTAG = 4
    DMA_CHUNKS = 4  # image load split into row-blocks
    CH_ROWS = (H + DMA_CHUNKS - 1) // DMA_CHUNKS

    with TileContext(nc) as tc:
        pid = nc.partition_id()
        with (
            tc.tile_pool(name="img", bufs=1) as img_pool,
            tc.tile_pool(name="wrk", bufs=1) as wrk_pool,
        ):
            G_ALLOC = 4
            img_tiles = [
                img_pool.tile([128, 2, HW], mybir.dt.float32, tag=f"img{g}", name=f"img{g}")
                for g in range(G_ALLOC)
            ]
            rowp = wrk_pool.tile([128, 2, OUT, W], mybir.dt.float32, tag="rowp")
            stags = [
                wrk_pool.tile([128, 8, 2, OUT * OUT], mybir.dt.float32, tag=f"stag{b}", name=f"stag{b}")
                for b in range(NSTAG)
            ]
            IMG_P = 2 * HW
            ROW_P = 2 * OUT * W
            STAG_P = 8 * 2 * OUT * OUT

            def emit_core(k):
                chunk = chunks[k]
                if len(chunk) == 0:
                    return
                groups = []
                g_of = {}
                for r in chunk:
                    n = int(ridx[r])
                    if n not in g_of:
                        g_of[n] = len(groups)
                        groups.append(n)
                for g, n in enumerate(groups):
                    # split the image load into row-chunks so the first
                    # reduces can start as soon as their rows land
                    for cb in range(DMA_CHUNKS):
                        h0 = cb * CH_ROWS
                        h1 = min(H, h0 + CH_ROWS)
                        if h0 >= h1:
                            continue
                        src = images_d[n, :, h0:h1].rearrange(
                            "(q p) h w -> p q (h w)", q=2
                        )
                        dst = bass.AP(
                            img_tiles[g % G_ALLOC].tensor,
                            img_tiles[g % G_ALLOC][:].offset + h0 * W,
                            [(IMG_P, 128), (HW, 2), (1, (h1 - h0) * W)],
                        )
                        nc.sync.dma_start(out=dst, in_=src)

                for idx, r in enumerate(chunk):
                    g = g_of[int(ridx[r])] % G_ALLOC
                    it = img_tiles[g]
                    X1, Y1, X2, Y2 = int(x1[r]), int(y1[r]), int(x2[r]), int(y2[r])
                    Hr, Wr = Y2 - Y1, X2 - X1
                    r8 = idx % 8
                    stag = stags[(idx // 8) % NSTAG]
                    if Hr <= 0 or Wr <= 0:
                        nc.gpsimd.memset(stag[:, r8], NEG)
                    else:
                        # ---- row phase: img -> rowp[q, i, 0:Wr]
                        for (i0, cnt, s0, stride, size) in _runs(_bins(Y1, Hr)):
                            in_ap = [(IMG_P, 128), (HW, 2)]
                            out_ap = [(ROW_P, 128), (OUT * W, 2)]
                            if cnt > 1:
                                in_ap.append((stride * W, cnt))
                                out_ap.append((W, cnt))
                            in_ap.append((1, Wr))
                            out_ap.append((1, Wr))
                            in_ap.append((W, size))
                            nc.vector.tensor_reduce(
                                bass.AP(rowp.tensor, rowp[:].offset + i0 * W, out_ap),
                                bass.AP(it.tensor, it[:].offset + s0 * W + X1, in_ap),
                                axis=mybir.AxisListType.X,
                                op=mybir.AluOpType.max,
                            )
                        # ---- col phase: rowp -> stag[r8][q, i, j]
                        for (j0, cnt, c0, stride, size) in _runs(_bins(0, Wr)):
                            in_ap = [(ROW_P, 128), (OUT * W, 2)]
                            out_ap = [(STAG_P, 128), (OUT * OUT, 2)]
                            if cnt > 1:
                                in_ap.append((stride, cnt))
                                out_ap.append((1, cnt))
                            in_ap.append((W, OUT))
                            out_ap.append((OUT, OUT))
                            in_ap.append((1, size))
                            nc.vector.tensor_reduce(
                                bass.AP(
                                    stag.tensor,
                                    stag[:].offset + r8 * 2 * OUT * OUT + j0,
                                    out_ap,
                                ),
                                bass.AP(rowp.tensor, rowp[:].offset + c0, in_ap),
                                axis=mybir.AxisListType.X,
                                op=mybir.AluOpType.max,
                            )
                    if r8 == 7 or idx == len(chunk) - 1:
                        cnt_r = r8 + 1
                        r0 = idx - r8
                        dst = out_d[r0 : r0 + cnt_r].rearrange(
                            "r (q p) i j -> p r q (i j)", q=2
                        )
                        nc.sync.dma_start(out=dst, in_=stag[:, 0:cnt_r])

            def cascade(k):
                if k == NCORES - 1:
                    emit_core(k)
                    return
                with tc.If(pid == k) as cmp:
                    emit_core(k)
                with cmp.Else():
                    cascade(k + 1)

            cascade(0)

    nc.compile()

    in_maps = [{"images": images} for _ in range(NCORES)]
    res = run_bass_kernel_spmd(nc, in_maps, list(range(NCORES)))

    full = np.empty((R, C, OUT, OUT), dtype=np.float32)
    for k in range(NCORES):
        ch = chunks[k]
        if len(ch):
            full[ch] = res.results[k]["out"][: len(ch)]
    return full


# revision 5
# speedup vs baseline: 1.0992x; 1.0992x over previous
"""ROIPool (adaptive max pool over ROI crops) for Trainium2, 8-core SPMD.

Strategy:
  - Host computes every ROI's crop geometry and adaptive-pool bin boundaries
    from the actual inputs (floor/ceil in float32, exactly as the reference).
  - ROIs are sorted by source image and dealt to the 8 cores in contiguous,
    cost-balanced chunks, so each core touches only 1-2 source images.
  - One SPMD Bass program is compiled per call.  Per-core work is baked as
    static instruction streams inside an If/Else-if cascade on
    partition_id (untaken blocks are jumped over entirely):
    each core DMAs its source image(s) into SBUF once ([128 partitions, 2
    channel-halves, 56*56], in 4 row-chunks so compute starts early) and
    then runs exact statically-sliced reduce_max instructions per ROI:
       row phase: for each run of row-bins with equal size+stride, one DVE
                  reduce over the h axis -> rowp[q, i, w_rel]
       col phase: for each run of col-bins, one DVE reduce over w -> out
    Outputs are staged 8 ROIs at a time (ring of 4 staging tiles) and
    DMA'd to DRAM.
  - Max in f32 is exact, so the result is bitwise identical to the
    reference regardless of reduction order.
"""

import numpy as np

OUT = 7
NCORES = 8
FIXED_ELEMS = 200  # ~fixed ns per DVE instr, in element-equivalents


# ---------------------------------------------------------------- host math
def _roi_geometry(images_shape, rois, roi_idx):
    """Replicates the reference's f32 floor/ceil box math on the host."""
    n, c, h, w = images_shape
    rois = np.asarray(rois, dtype=np.float32)
    roi_idx = np.asarray(roi_idx).astype(np.int64)
    x1 = np.floor(rois[:, 0] * np.float32(w)).astype(np.int64)
    y1 = np.floor(rois[:, 1] * np.float32(h)).astype(np.int64)
    x2 = np.ceil(rois[:, 2] * np.float32(w)).astype(np.int64)
    y2 = np.ceil(rois[:, 3] * np.float32(h)).astype(np.int64)
    return x1, y1, x2, y2, roi_idx


def _bins(start, length):
    """Adaptive pool bins [start + i*L//OUT, start + ceil((i+1)*L/OUT)) ."""
    out = []
    for i in range(OUT):
        s = start + (i * length) // OUT
        e = start + ((i + 1) * length + OUT - 1) // OUT
        out.append((int(s), int(e)))
    return out


def _runs(bins):
    """Group consecutive bins into maximal runs with equal size and equal
    start stride.  Returns list of (i0, count, s0, stride, size)."""
    runs = []
    i = 0
    while i < OUT:
        s0, e0 = bins[i]
        size = e0 - s0
        j = i + 1
        stride = None
        while j < OUT:
            s, e = bins[j]
            if e - s != size:
                break
            st = s - bins[j - 1][0]
            if stride is None:
                stride = st
            elif st != stride:
                break
            j += 1
        runs.append((i, j - i, s0, 0 if stride is None else stride, size))
        i = j
    return runs


def _roi_cost(Hr, Wr, y1):
    """Estimated DVE cost (element-equivalents) of one ROI."""
    if Hr <= 0 or Wr <= 0:
        return FIXED_ELEMS
    rr = _runs(_bins(y1, Hr))
    cr = _runs(_bins(0, Wr))
    row = sum(2 * cnt * sz * Wr for (_, cnt, _, _, sz) in rr)
    col = sum(2 * cnt * OUT * sz for (_, cnt, _, _, sz) in cr)
    return row + col + FIXED_ELEMS * (len(rr) + len(cr))


def _balanced_split(costs, k):
    """Split range(len(costs)) into k contiguous chunks with near-equal
    total cost (prefix-sum fractional boundaries)."""
    c = np.asarray(costs, dtype=np.float64)
    pref = np.concatenate([[0.0], np.cumsum(c)])
    total = pref[-1]
    bounds = [0]
    for i in range(1, k):
        bounds.append(int(np.searchsorted(pref, total * i / k)))
    bounds.append(len(costs))
    # enforce monotonicity
    for i in range(1, len(bounds)):
        bounds[i] = max(bounds[i], bounds[i - 1])
    return [(bounds[i], bounds[i + 1]) for i in range(k)]


# ---------------------------------------------------------------- kernel
def kernel(images, rois, roi_idx):
    import concourse.bacc as bacc
    import concourse.bass as bass
    import concourse.mybir as mybir
    from concourse.bass_utils import run_bass_kernel_spmd
    from concourse.tile import TileContext

    images = np.ascontiguousarray(np.asarray(images, dtype=np.float32))
    N, C, H, W = images.shape
    R = np.asarray(rois).shape[0]
    assert C == 256 and H == 56 and W == 56, "kernel hardcoded for C=256, H=W=56"
    HW = H * W
    x1, y1, x2, y2, ridx = _roi_geometry(images.shape, rois, roi_idx)

    # order ROIs by source image (then by bottom row, so earlier image
    # row-chunks unblock the first ROIs), cost-balanced contiguous chunks
    order = sorted(range(R), key=lambda r: (int(ridx[r]), int(y2[r]), r))
    costs = [_roi_cost(int(y2[r] - y1[r]), int(x2[r] - x1[r]), int(y1[r])) for r in order]
    chunks = [np.array(order[a:b], dtype=np.int64) for a, b in _balanced_split(costs, NCORES)]
    nr_max = max(len(ch) for ch in chunks)

    nc = bacc.Bacc("TRN2", target_bir_lowering=False, debug=False, num_devices=NCORES)
    images_d = nc.dram_tensor("images", [N, C, H, W], mybir.dt.float32, kind="ExternalInput")
    out_d = nc.dram_tensor("out", [max(nr_max, 1), C, OUT, OUT], mybir.dt.float32, kind="ExternalOutput")

    NEG = float(np.finfo(np.float32).min)
    NSTAG = 4
    DMA_CHUNKS = 4  # image load split into row-blocks
    CH_ROWS = (H + DMA_CHUNKS - 1) // DMA_CHUNKS

    with TileContext(nc) as tc:
        # only DVE (reduces) and SP (DMAs) execute inside the per-core
        # branches — restricting the If to those engines keeps the other
        # three engines branch-free and makes block entry/exit much cheaper
        pid = nc.partition_id(engines=(mybir.EngineType.DVE, mybir.EngineType.SP))
        with (
            tc.tile_pool(name="img", bufs=1) as img_pool,
            tc.tile_pool(name="wrk", bufs=1) as wrk_pool,
        ):
            G_ALLOC = 4
            img_tiles = [
                img_pool.tile([128, 2, HW], mybir.dt.float32, tag=f"img{g}", name=f"img{g}")
                for g in range(G_ALLOC)
            ]
            rowp = wrk_pool.tile([128, 2, OUT, W], mybir.dt.float32, tag="rowp")
            stags = [
                wrk_pool.tile([128, 8, 2, OUT * OUT], mybir.dt.float32, tag=f"stag{b}", name=f"stag{b}")
                for b in range(NSTAG)
            ]
            IMG_P = 2 * HW
            ROW_P = 2 * OUT * W
            STAG_P = 8 * 2 * OUT * OUT

            def emit_core(k):
                chunk = chunks[k]
                if len(chunk) == 0:
                    return
                groups = []
                g_of = {}
                for r in chunk:
                    n = int(ridx[r])
                    if n not in g_of:
                        g_of[n] = len(groups)
                        groups.append(n)
                for g, n in enumerate(groups):
                    # split the image load into row-chunks so the first
                    # reduces can start as soon as their rows land
                    for cb in range(DMA_CHUNKS):
                        h0 = cb * CH_ROWS
                        h1 = min(H, h0 + CH_ROWS)
                        if h0 >= h1:
                            continue
                        src = images_d[n, :, h0:h1].rearrange(
                            "(q p) h w -> p q (h w)", q=2
                        )
                        dst = bass.AP(
                            img_tiles[g % G_ALLOC].tensor,
                            img_tiles[g % G_ALLOC][:].offset + h0 * W,
                            [(IMG_P, 128), (HW, 2), (1, (h1 - h0) * W)],
                        )
                        nc.sync.dma_start(out=dst, in_=src)

                for idx, r in enumerate(chunk):
                    g = g_of[int(ridx[r])] % G_ALLOC
                    it = img_tiles[g]
                    X1, Y1, X2, Y2 = int(x1[r]), int(y1[r]), int(x2[r]), int(y2[r])
                    Hr, Wr = Y2 - Y1, X2 - X1
                    r8 = idx % 8
                    stag = stags[(idx // 8) % NSTAG]
                    if Hr > 0 and Wr > 0:
                        # ---- row phase: img -> rowp[q, i, 0:Wr]
                        for (i0, cnt, s0, stride, size) in _runs(_bins(Y1, Hr)):
                            in_ap = [(IMG_P, 128), (HW, 2)]
                            out_ap = [(ROW_P, 128), (OUT * W, 2)]
                            if cnt > 1:
                                in_ap.append((stride * W, cnt))
                                out_ap.append((W, cnt))
                            in_ap.append((1, Wr))
                            out_ap.append((1, Wr))
                            in_ap.append((W, size))
                            nc.vector.tensor_reduce(
                                bass.AP(rowp.tensor, rowp[:].offset + i0 * W, out_ap),
                                bass.AP(it.tensor, it[:].offset + s0 * W + X1, in_ap),
                                axis=mybir.AxisListType.X,
                                op=mybir.AluOpType.max,
                            )
                        # ---- col phase: rowp -> stag[r8][q, i, j]
                        for (j0, cnt, c0, stride, size) in _runs(_bins(0, Wr)):
                            in_ap = [(ROW_P, 128), (OUT * W, 2)]
                            out_ap = [(STAG_P, 128), (OUT * OUT, 2)]
                            if cnt > 1:
                                in_ap.append((stride, cnt))
                                out_ap.append((1, cnt))
                            in_ap.append((W, OUT))
                            out_ap.append((OUT, OUT))
                            in_ap.append((1, size))
                            nc.vector.tensor_reduce(
                                bass.AP(
                                    stag.tensor,
                                    stag[:].offset + r8 * 2 * OUT * OUT + j0,
                                    out_ap,
                                ),
                                bass.AP(rowp.tensor, rowp[:].offset + c0, in_ap),
                                axis=mybir.AxisListType.X,
                                op=mybir.AluOpType.max,
                            )
                    if r8 == 7 or idx == len(chunk) - 1:
                        cnt_r = r8 + 1
                        r0 = idx - r8
                        dst = out_d[r0 : r0 + cnt_r].rearrange(
                            "r (q p) i j -> p r q (i j)", q=2
                        )
                        nc.sync.dma_start(out=dst, in_=stag[:, 0:cnt_r])

            def tree(lo, hi):
                if hi - lo == 1:
                    emit_core(lo)
                    return
                mid = (lo + hi) // 2
                with tc.If(pid < mid) as cmp:
                    tree(lo, mid)
                with cmp.Else():
                    tree(mid, hi)

            tree(0, NCORES)

    nc.compile()

    in_maps = [{"images": images} for _ in range(NCORES)]
    res = run_bass_kernel_spmd(nc, in_maps, list(range(NCORES)))

    full = np.empty((R, C, OUT, OUT), dtype=np.float32)
    for k in range(NCORES):
        ch = chunks[k]
        if len(ch):
            full[ch] = res.results[k]["out"][: len(ch)]
    degen = (y2 - y1 <= 0) | (x2 - x1 <= 0)
    if degen.any():
        full[degen] = NEG
    return full
